# revision 6
# baseline (speedup 1.0000x reference)
"""Block-descriptor variant: the coef diagonals are written as 16x16
diagonal blocks -> every DMA descriptor is a 64-B-aligned 64-B row
(~1.75 ns/pkt drain vs ~10 ns/pkt for 4-B descriptors).

Layout ("D2"): SBUF tiles [128, 64] hold input element (b, i) with
  j = i // 16, r = i % 16, o = b % 2, m = b // 2
  partition q = 64*o + j,  free f = 16*m + r
so the per-matrix staging scatter and the block-row DMA sources are all
partition-aligned.
"""
import numpy as np

import bass_rust
import concourse.bacc as bacc
import concourse.mybir as mybir
import concourse.tile as tile
from concourse.bass_utils import run_bass_kernel_spmd

B, N = 64, 1024
NCORES = 8
BL = B // NCORES  # 8
EPS = 1e-8
P = 128
F = BL * N // P   # 64
DT = mybir.dt.float32

_module_cache = {}


def _raw(base, pairs):
    c = base.copy()
    c.ap = bass_rust.VecI64Pair(pairs)
    return c


def _d2(dram_2d, o):
    """[BL, N] dram tensor -> 3D AP (j, m, r) for batch-parity o."""
    ap = _raw(dram_2d[:].flatten(), [[16, 64], [2048, 4], [1, 16]])
    ap.offset = ap.offset + 1024 * o
    return ap


def build_module():
    nc = bacc.Bacc("TRN2", target_bir_lowering=False, debug=False)

    lo_d = nc.dram_tensor("lower", [BL, N], DT, kind="ExternalInput")
    up_d = nc.dram_tensor("upper", [BL, N], DT, kind="ExternalInput")
    cl_d = nc.dram_tensor("concrete_lower", [BL, N], DT, kind="ExternalOutput")
    cu_d = nc.dram_tensor("concrete_upper", [BL, N], DT, kind="ExternalOutput")
    lc_d = nc.dram_tensor("lower_coef", [BL, N, N], DT, kind="ExternalOutput")
    uc_d = nc.dram_tensor("upper_coef", [BL, N, N], DT, kind="ExternalOutput")
    lb_d = nc.dram_tensor("lower_bias", [BL, N], DT, kind="ExternalOutput")
    ub_d = nc.dram_tensor("upper_bias", [BL, N], DT, kind="ExternalOutput")

    with tile.TileContext(nc) as tc:
        with tc.tile_pool(name="sbuf", bufs=1) as pool:
            # Staging tiles for the diagonal blocks; memsets first (no deps).
            s_lc = pool.tile([P, 1024], DT)
            s_uc = pool.tile([P, 1024], DT)
            zzt = pool.tile([P, F], DT)
            nc.vector.memset(s_lc[:], 0.0)
            nc.gpsimd.memset(s_uc[:], 0.0)
            nc.vector.memset(zzt[:], 0.0)

            lo = pool.tile([P, F], DT)
            up = pool.tile([P, F], DT)
            nc.sync.dma_start(lo[:64, :], _d2(lo_d, 0))
            nc.sync.dma_start(lo[64:, :], _d2(lo_d, 1))
            nc.scalar.dma_start(up[:64, :], _d2(up_d, 0))
            nc.scalar.dma_start(up[64:, :], _d2(up_d, 1))

            cl = pool.tile([P, F], DT)
            cu = pool.tile([P, F], DT)
            a = pool.tile([P, F], DT)
            g = pool.tile([P, F], DT)
            den = pool.tile([P, F], DT)
            lam = pool.tile([P, F], DT)
            na = pool.tile([P, F], DT)
            mu = pool.tile([P, F], DT)
            ud = pool.tile([P, F], DT)

            def scatter(s_tile, d_tile):
                # S[q, 256*m + 17*r] = d[q, 16*m + r]
                dst = s_tile[:].rearrange("p (m x) -> p m x", x=256)[:, :, ::17]
                src = d_tile[:].rearrange("p (m r) -> p m r", r=16)
                nc.vector.tensor_copy(dst, src)

            def diag_blocks(coef_d, s_tile, eng):
                # one DMA per matrix: 1024 descriptors x 64 B
                for b in range(BL):
                    o, m = b % 2, b // 2
                    src = s_tile[64 * o:64 * o + 64, 256 * m:256 * m + 256] \
                        .rearrange("p (r c) -> p r c", c=16)
                    dst = _raw(coef_d[b].flatten(),
                               [[16400, 64], [1024, 16], [1, 16]])
                    eng.dma_start(dst, src)

            nc.vector.tensor_scalar(a[:], lo[:], 0.0, None, mybir.AluOpType.is_ge)
            scatter(s_lc, a)
            diag_blocks(lc_d, s_lc, nc.sync)

            nc.vector.tensor_sub(den[:], up[:], lo[:])
            nc.vector.tensor_scalar_add(den[:], den[:], EPS)
            nc.vector.reciprocal(den[:], den[:])
            nc.vector.tensor_mul(lam[:], up[:], den[:])
            nc.vector.tensor_scalar(g[:], up[:], 0.0, None, mybir.AluOpType.is_gt)
            nc.vector.tensor_scalar(na[:], a[:], -1.0, 1.0,
                                    mybir.AluOpType.mult, mybir.AluOpType.add)
            nc.vector.tensor_mul(na[:], g[:], na[:])     # crossing
            nc.vector.tensor_mul(lam[:], lam[:], na[:])  # lambda_upper
            nc.vector.tensor_add(ud[:], a[:], lam[:])    # upper diag values
            scatter(s_uc, ud)
            diag_blocks(uc_d, s_uc, nc.scalar)

            nc.vector.tensor_scalar_max(cl[:], lo[:], 0.0)
            nc.vector.tensor_scalar_max(cu[:], up[:], 0.0)
            nc.vector.tensor_mul(mu[:], lam[:], lo[:])
            nc.vector.tensor_scalar_mul(mu[:], mu[:], -1.0)

            # Small outputs: D2-layout tiles go back via the same permuted
            # 3D APs; lower_bias is all zeros so a contiguous write is fine.
            nc.sync.dma_start(_d2(ub_d, 0), mu[:64, :])
            nc.sync.dma_start(_d2(ub_d, 1), mu[64:, :])
            nc.scalar.dma_start(_d2(cl_d, 0), cl[:64, :])
            nc.scalar.dma_start(_d2(cl_d, 1), cl[64:, :])
            nc.scalar.dma_start(_d2(cu_d, 0), cu[:64, :])
            nc.scalar.dma_start(_d2(cu_d, 1), cu[64:, :])
            nc.sync.dma_start(lb_d[:].flatten().rearrange("(p f) -> p f", p=P),
                              zzt[:])

    nc.finalize()
    return nc


def _get_module():
    if "nc" not in _module_cache:
        _module_cache["nc"] = build_module()
    return _module_cache["nc"]


def kernel(lower: np.ndarray, upper: np.ndarray):
    lower = np.ascontiguousarray(lower, dtype=np.float32)
    upper = np.ascontiguousarray(upper, dtype=np.float32)
    nc = _get_module()
    in_maps = [
        {"lower": lower[c * BL:(c + 1) * BL], "upper": upper[c * BL:(c + 1) * BL]}
        for c in range(NCORES)
    ]
    res = run_bass_kernel_spmd(nc, in_maps, list(range(NCORES)))
    outs = res.results

    def gather(name):
        return np.concatenate([outs[c][name] for c in range(NCORES)], axis=0)

    return (
        gather("concrete_lower"),
        gather("concrete_upper"),
        gather("lower_coef"),
        gather("upper_coef"),
        gather("lower_bias"),
        gather("upper_bias"),
    )
